# revision 1
# baseline (speedup 1.0000x reference)
"""Divergence-free RBF kernel Gram matrix on 8 Trainium2 NeuronCores.

Math: for d=2, with scaled coords x' = x*exp(-ll/2):
  dx = x0_i - y0_j, dy = x1_i - y1_j, r2 = dx^2 + dy^2, e = exp(-r2/2)
  K[2i+0, 2j+0] = e * (1 - dy^2)
  K[2i+0, 2j+1] = K[2i+1, 2j+0] = e * dx*dy
  K[2i+1, 2j+1] = e * (1 - dx^2)

Each polynomial factor is low-rank in the basis {1, x0, x1, x0*x1, x0^2, x1^2}
(K=6): host precomputes L [6, n] (X side) and column-interleaved R [6, 2m]
(Y side), device builds the polynomial matrices with PE matmuls, exp on ACT,
and one DVE multiply per output element. fp32-grade matmul precision comes
from a hi/lo bf16 split stacked to K=18: [Lhi;Llo;Lhi].T @ [Rhi;Rhi;Rlo].

Sharding: rows of X (n axis) split across 8 cores, 512 each -> each core
writes 1024 output rows of the (8192, 8192) Gram matrix. No communication.
"""

import numpy as np
import ml_dtypes

N = 4096          # X rows
M = 4096          # Y rows
D = 2
NCORES = 8
NPC = N // NCORES  # 512 X rows per core
IB = 128           # i-block = partition count
NIB = NPC // IB    # 4 i-blocks per core
JG = 256           # j-group size (j count per PSUM tile)
NJG = M // JG      # 16 j-groups
KST = 18           # stacked contraction dim (3 x 6 basis rows)

_cache = {}


def _hi_lo(a):
    bf = ml_dtypes.bfloat16
    hi = a.astype(bf)
    lo = (a - hi.astype(np.float64)).astype(bf)
    return hi, lo


def _prepare_inputs(X, Y, log_length_scale):
    s = float(np.exp(-0.5 * np.float64(np.asarray(log_length_scale).reshape(-1)[0])))
    xs = np.asarray(X, dtype=np.float64).reshape(N, D) * s
    ys = np.asarray(Y, dtype=np.float64).reshape(M, D) * s
    x0, x1 = xs[:, 0], xs[:, 1]
    y0, y1 = ys[:, 0], ys[:, 1]
    one_n, zero_m, one_m = np.ones(N), np.zeros(M), np.ones(M)

    # X-side basis [6, N]: rows {1, x0, x1, x0*x1, x0^2, x1^2}
    L = np.stack([one_n, x0, x1, x0 * x1, x0 ** 2, x1 ** 2])

    # Y-side coefficient columns [6, M] per output channel
    c_dxdy = np.stack([y0 * y1, -y1, -y0, one_m, zero_m, zero_m])
    c_00 = np.stack([1 - y1 ** 2, zero_m, 2 * y1, zero_m, zero_m, -one_m])
    c_11 = np.stack([1 - y0 ** 2, 2 * y0, zero_m, zero_m, -one_m, zero_m])
    c_r2 = np.stack([y0 ** 2 + y1 ** 2, -2 * y0, -2 * y1, zero_m, one_m, one_m])

    Re = np.zeros((6, 2 * M))   # even output rows: [1-dy^2 | dxdy] interleaved
    Re[:, 0::2] = c_00
    Re[:, 1::2] = c_dxdy
    Ro = np.zeros((6, 2 * M))   # odd output rows: [dxdy | 1-dx^2] interleaved
    Ro[:, 0::2] = c_dxdy
    Ro[:, 1::2] = c_11

    # Merge Re/Ro into one tensor so each j-group is a single N=1024 matmul:
    # group g occupies cols [1024g, 1024g+1024) = [Re_g (512) | Ro_g (512)]
    Reo = np.zeros((6, 4 * M))
    v = Reo.reshape(6, 2 * M // 512, 2, 512)
    v[:, :, 0, :] = Re.reshape(6, -1, 512)
    v[:, :, 1, :] = Ro.reshape(6, -1, 512)

    Lh, Ll = _hi_lo(L)
    Lst = np.ascontiguousarray(np.concatenate([Lh, Ll, Lh], axis=0))  # (18, N)

    def r_stack(R):
        Rh, Rl = _hi_lo(R)
        return np.ascontiguousarray(np.concatenate([Rh, Rh, Rl], axis=0))

    return Lst, r_stack(Reo), r_stack(c_r2)


def _build_module(bass_cls=None, reps=1, **bass_kw):
    from concourse import bacc, mybir
    import concourse.tile as tile

    bf16 = mybir.dt.bfloat16
    f32 = mybir.dt.float32
    Exp = mybir.ActivationFunctionType.Exp

    if bass_cls is None:
        bass_cls = bacc.Bacc
    nc = bass_cls("TRN2", target_bir_lowering=False, debug=False,
                  enable_asserts=False, **bass_kw)
    lhsT_d = nc.dram_tensor("lhsT", [KST, NPC], bf16, kind="ExternalInput")
    reo_d = nc.dram_tensor("r_eo", [KST, 4 * M], bf16, kind="ExternalInput")
    rr_d = nc.dram_tensor("r_r2", [KST, M], bf16, kind="ExternalInput")
    out_d = nc.dram_tensor("out", [2 * NPC, 2 * M], f32, kind="ExternalOutput")

    QJ = 4 * JG  # 1024 j's covered by one r2/exp quad

    with tile.TileContext(nc) as tc:
        with (
            tc.tile_pool(name="const", bufs=1) as cpool,
            tc.tile_pool(name="outp", bufs=2) as opool,
            tc.tile_pool(name="ep", bufs=3) as epool,
            tc.tile_pool(name="ps", bufs=2, space="PSUM") as ppool,
        ):
            lhsT = cpool.tile([KST, NPC], bf16)
            nc.sync.dma_start(out=lhsT[:], in_=lhsT_d[:, :])
            reo_sb = cpool.tile([KST, 4 * M], bf16)
            nc.sync.dma_start(out=reo_sb[:], in_=reo_d[:, :])
            rr_sb = cpool.tile([KST, M], bf16)
            nc.sync.dma_start(out=rr_sb[:], in_=rr_d[:, :])

            out_view = out_d.ap().rearrange("(i t) c -> i t c", t=2)

            for ib in [i for _ in range(reps) for i in range(NIB)]:
                wt = lhsT[:, ib * IB:(ib + 1) * IB]
                # halves: [0:8192) even output rows, [8192:16384) odd rows
                out_all = opool.tile([IB, 4 * M], f32, tag="out_all")
                out_q = out_all[:].rearrange("p (h j t) -> p h j t", h=2, t=2)
                for q in range(M // QJ):
                    r2q = ppool.tile([IB, QJ], f32, tag="r2")
                    for s in range(QJ // 512):
                        nc.tensor.matmul(
                            r2q[:, s * 512:(s + 1) * 512], wt,
                            rr_sb[:, q * QJ + s * 512:q * QJ + (s + 1) * 512],
                            start=True, stop=True)
                    ebig = epool.tile([IB, QJ], f32, tag="e")
                    nc.scalar.activation(ebig[:], r2q[:], Exp, scale=-0.5)
                    for h in range(QJ // JG):
                        g = q * (QJ // JG) + h
                        memo = ppool.tile([IB, 4 * JG], f32, tag="memo")
                        for s in range(4 * JG // 512):
                            nc.tensor.matmul(
                                memo[:, s * 512:(s + 1) * 512], wt,
                                reo_sb[:, g * 4 * JG + s * 512:
                                       g * 4 * JG + (s + 1) * 512],
                                start=True, stop=True)
                        eb = (ebig[:, h * JG:(h + 1) * JG]
                              .unsqueeze(1).unsqueeze(3)
                              .broadcast_to([IB, 2, JG, 2]))
                        nc.vector.tensor_mul(
                            out_q[:, :, g * JG:(g + 1) * JG, :],
                            memo[:].rearrange("p (h j t) -> p h j t", h=2, t=2),
                            eb,
                        )
                i0 = ib * IB
                nc.sync.dma_start(out=out_view[i0:i0 + IB, 0:1, :].squeeze(1),
                                  in_=out_all[:, 0:2 * M])
                nc.sync.dma_start(out=out_view[i0:i0 + IB, 1:2, :].squeeze(1),
                                  in_=out_all[:, 2 * M:4 * M])
    nc.finalize()
    return nc


def _run(X, Y, log_length_scale, trace=False):
    from concourse.bass_utils import run_bass_kernel_spmd

    Lst, Reo, Rr = _prepare_inputs(X, Y, log_length_scale)
    if "nc" not in _cache:
        _cache["nc"] = _build_module()
    nc = _cache["nc"]
    in_maps = [
        {
            "lhsT": np.ascontiguousarray(Lst[:, c * NPC:(c + 1) * NPC]),
            "r_eo": Reo,
            "r_r2": Rr,
        }
        for c in range(NCORES)
    ]
    res = run_bass_kernel_spmd(nc, in_maps, core_ids=list(range(NCORES)),
                               trace=trace)
    out = np.concatenate([r["out"] for r in res.results], axis=0)
    return out.reshape(1, 2 * N, 2 * M), res


def kernel(X, Y, log_length_scale):
    out, _ = _run(np.asarray(X), np.asarray(Y), np.asarray(log_length_scale))
    return out



# revision 2
# speedup vs baseline: 1.0483x; 1.0483x over previous
"""Divergence-free RBF kernel Gram matrix on 8 Trainium2 NeuronCores.

Math: for d=2, with scaled coords x' = x*exp(-ll/2):
  dx = x0_i - y0_j, dy = x1_i - y1_j, r2 = dx^2 + dy^2, e = exp(-r2/2)
  K[2i+0, 2j+0] = e * (1 - dy^2)
  K[2i+0, 2j+1] = K[2i+1, 2j+0] = e * dx*dy
  K[2i+1, 2j+1] = e * (1 - dx^2)

Each polynomial factor is low-rank in the basis {1, x0, x1, x0*x1, x0^2, x1^2}
(K=6): host precomputes L [6, n] (X side) and column-interleaved R [6, 2m]
(Y side), device builds the polynomial matrices with PE matmuls, exp on ACT,
and one DVE multiply per output element. fp32-grade matmul precision comes
from a hi/lo bf16 split stacked to K=18: [Lhi;Llo;Lhi].T @ [Rhi;Rhi;Rlo].

Sharding: rows of X (n axis) split across 8 cores, 512 each -> each core
writes 1024 output rows of the (8192, 8192) Gram matrix. No communication.
"""

import numpy as np
import ml_dtypes

N = 4096          # X rows
M = 4096          # Y rows
D = 2
NCORES = 8
NPC = N // NCORES  # 512 X rows per core
IB = 128           # i-block = partition count
NIB = NPC // IB    # 4 i-blocks per core
JG = 256           # j-group size (j count per PSUM tile)
NJG = M // JG      # 16 j-groups
KST = 18           # stacked contraction dim (3 x 6 basis rows)

_cache = {}


def _hi_lo(a):
    bf = ml_dtypes.bfloat16
    hi = a.astype(bf)
    lo = (a - hi.astype(np.float64)).astype(bf)
    return hi, lo


def _prepare_inputs(X, Y, log_length_scale):
    s = float(np.exp(-0.5 * np.float64(np.asarray(log_length_scale).reshape(-1)[0])))
    xs = np.asarray(X, dtype=np.float64).reshape(N, D) * s
    ys = np.asarray(Y, dtype=np.float64).reshape(M, D) * s
    x0, x1 = xs[:, 0], xs[:, 1]
    y0, y1 = ys[:, 0], ys[:, 1]
    one_n, zero_m, one_m = np.ones(N), np.zeros(M), np.ones(M)

    # X-side basis [6, N]: rows {1, x0, x1, x0*x1, x0^2, x1^2}
    L = np.stack([one_n, x0, x1, x0 * x1, x0 ** 2, x1 ** 2])

    # Y-side coefficient columns [6, M] per output channel
    c_dxdy = np.stack([y0 * y1, -y1, -y0, one_m, zero_m, zero_m])
    c_00 = np.stack([1 - y1 ** 2, zero_m, 2 * y1, zero_m, zero_m, -one_m])
    c_11 = np.stack([1 - y0 ** 2, 2 * y0, zero_m, zero_m, -one_m, zero_m])
    c_r2 = np.stack([y0 ** 2 + y1 ** 2, -2 * y0, -2 * y1, zero_m, one_m, one_m])

    Re = np.zeros((6, 2 * M))   # even output rows: [1-dy^2 | dxdy] interleaved
    Re[:, 0::2] = c_00
    Re[:, 1::2] = c_dxdy
    Ro = np.zeros((6, 2 * M))   # odd output rows: [dxdy | 1-dx^2] interleaved
    Ro[:, 0::2] = c_dxdy
    Ro[:, 1::2] = c_11

    # Merge Re/Ro into one tensor so each j-group is a single N=1024 matmul:
    # group g occupies cols [1024g, 1024g+1024) = [Re_g (512) | Ro_g (512)]
    Reo = np.zeros((6, 4 * M))
    v = Reo.reshape(6, 2 * M // 512, 2, 512)
    v[:, :, 0, :] = Re.reshape(6, -1, 512)
    v[:, :, 1, :] = Ro.reshape(6, -1, 512)

    Lh, Ll = _hi_lo(L)
    Lst = np.ascontiguousarray(np.concatenate([Lh, Ll, Lh], axis=0))  # (18, N)

    def r_stack(R):
        Rh, Rl = _hi_lo(R)
        return np.ascontiguousarray(np.concatenate([Rh, Rh, Rl], axis=0))

    return Lst, r_stack(Reo), r_stack(c_r2)


def _build_module(bass_cls=None, reps=1, **bass_kw):
    from concourse import bacc, mybir
    import concourse.tile as tile

    bf16 = mybir.dt.bfloat16
    f32 = mybir.dt.float32
    Exp = mybir.ActivationFunctionType.Exp

    if bass_cls is None:
        bass_cls = bacc.Bacc
    nc = bass_cls("TRN2", target_bir_lowering=False, debug=False,
                  enable_asserts=False, **bass_kw)
    lhsT_d = nc.dram_tensor("lhsT", [KST, NPC], bf16, kind="ExternalInput")
    reo_d = nc.dram_tensor("r_eo", [KST, 4 * M], bf16, kind="ExternalInput")
    rr_d = nc.dram_tensor("r_r2", [KST, M], bf16, kind="ExternalInput")
    out_d = nc.dram_tensor("out", [2 * NPC, 2 * M], f32, kind="ExternalOutput")

    QJ = 4 * JG  # 1024 j's covered by one r2/exp quad
    NQ = M // QJ  # 4 quads

    with tile.TileContext(nc) as tc:
        with (
            tc.tile_pool(name="const", bufs=1) as cpool,
            tc.tile_pool(name="outp", bufs=4) as opool,
            tc.tile_pool(name="ep", bufs=3) as epool,
            tc.tile_pool(name="ps", bufs=2, space="PSUM") as ppool,
        ):
            # Load order matters: lhsT+rr unblock the q=0 r2 matmul at ~2us;
            # reo arrives in q-sized chunks so memo matmuls start early too.
            lhsT = cpool.tile([KST, NPC], bf16)
            nc.sync.dma_start(out=lhsT[:], in_=lhsT_d[:, :])
            rr_sb = cpool.tile([KST, M], bf16)
            nc.sync.dma_start(out=rr_sb[:], in_=rr_d[:, :])
            reo_sb = []
            for qq in range(NQ):
                t = cpool.tile([KST, 4 * QJ], bf16, tag=f"reo{qq}")
                nc.sync.dma_start(
                    out=t[:], in_=reo_d[:, qq * 4 * QJ:(qq + 1) * 4 * QJ])
                reo_sb.append(t)

            out_view = out_d.ap().rearrange("(i t) c -> i t c", t=2)

            for ib in [i for _ in range(reps) for i in range(NIB)]:
                wt = lhsT[:, ib * IB:(ib + 1) * IB]
                i0 = ib * IB
                for q in range(NQ):
                    r2q = ppool.tile([IB, QJ], f32, tag="r2")
                    for s in range(QJ // 512):
                        nc.tensor.matmul(
                            r2q[:, s * 512:(s + 1) * 512], wt,
                            rr_sb[:, q * QJ + s * 512:q * QJ + (s + 1) * 512],
                            start=True, stop=True)
                    ebig = epool.tile([IB, QJ], f32, tag="e")
                    nc.scalar.activation(ebig[:], r2q[:], Exp, scale=-0.5)
                    # per-quad output tile: cols (h, j_local, t), h = even/odd
                    oq = opool.tile([IB, 4 * QJ], f32, tag="oq")
                    oq4 = oq[:].rearrange("p (h j t) -> p h j t", h=2, t=2)
                    for h in range(QJ // JG):
                        memo = ppool.tile([IB, 4 * JG], f32, tag="memo")
                        for s in range(4 * JG // 512):
                            nc.tensor.matmul(
                                memo[:, s * 512:(s + 1) * 512], wt,
                                reo_sb[q][:, h * 4 * JG + s * 512:
                                           h * 4 * JG + (s + 1) * 512],
                                start=True, stop=True)
                        eb = (ebig[:, h * JG:(h + 1) * JG]
                              .unsqueeze(1).unsqueeze(3)
                              .broadcast_to([IB, 2, JG, 2]))
                        nc.vector.tensor_mul(
                            oq4[:, :, h * JG:(h + 1) * JG, :],
                            memo[:].rearrange("p (h j t) -> p h j t", h=2, t=2),
                            eb,
                        )
                    # stream this 2MB quad out immediately (both row halves)
                    nc.sync.dma_start(
                        out=out_view[i0:i0 + IB, :, q * 2 * QJ:(q + 1) * 2 * QJ],
                        in_=oq[:].rearrange("p (h c) -> p h c", h=2))
    nc.finalize()
    return nc


def _run(X, Y, log_length_scale, trace=False):
    from concourse.bass_utils import run_bass_kernel_spmd

    Lst, Reo, Rr = _prepare_inputs(X, Y, log_length_scale)
    if "nc" not in _cache:
        _cache["nc"] = _build_module()
    nc = _cache["nc"]
    in_maps = [
        {
            "lhsT": np.ascontiguousarray(Lst[:, c * NPC:(c + 1) * NPC]),
            "r_eo": Reo,
            "r_r2": Rr,
        }
        for c in range(NCORES)
    ]
    res = run_bass_kernel_spmd(nc, in_maps, core_ids=list(range(NCORES)),
                               trace=trace)
    out = np.concatenate([r["out"] for r in res.results], axis=0)
    return out.reshape(1, 2 * N, 2 * M), res


def kernel(X, Y, log_length_scale):
    out, _ = _run(np.asarray(X), np.asarray(Y), np.asarray(log_length_scale))
    return out



# revision 5
# speedup vs baseline: 1.0875x; 1.0375x over previous
"""Divergence-free RBF kernel Gram matrix on 8 Trainium2 NeuronCores.

Math: for d=2, with scaled coords x' = x*exp(-ll/2):
  dx = x0_i - y0_j, dy = x1_i - y1_j, r2 = dx^2 + dy^2, e = exp(-r2/2)
  K[2i+0, 2j+0] = e * (1 - dy^2)
  K[2i+0, 2j+1] = K[2i+1, 2j+0] = e * dx*dy
  K[2i+1, 2j+1] = e * (1 - dx^2)

Each polynomial factor is low-rank in the basis {1, x0, x1, x0*x1, x0^2, x1^2}
(K=6): host precomputes L [6, n] (X side) and column-interleaved R [6, 2m]
(Y side), device builds the polynomial matrices with PE matmuls, exp on ACT,
and one DVE multiply per output element. fp32-grade matmul precision comes
from a hi/lo bf16 split stacked to K=18: [Lhi;Llo;Lhi].T @ [Rhi;Rhi;Rlo].

Sharding: rows of X (n axis) split across 8 cores, 512 each -> each core
writes 1024 output rows of the (8192, 8192) Gram matrix. No communication.
"""

import numpy as np
import ml_dtypes

N = 4096          # X rows
M = 4096          # Y rows
D = 2
NCORES = 8
NPC = N // NCORES  # 512 X rows per core
IB = 128           # i-block = partition count
NIB = NPC // IB    # 4 i-blocks per core
JG = 256           # j-group size (j count per PSUM tile)
NJG = M // JG      # 16 j-groups
KST = 18           # stacked contraction dim (3 x 6 basis rows)

_cache = {}


def _hi_lo(a):
    bf = ml_dtypes.bfloat16
    hi = a.astype(bf)
    lo = (a - hi.astype(np.float64)).astype(bf)
    return hi, lo


def _prepare_inputs(X, Y, log_length_scale):
    s = float(np.exp(-0.5 * np.float64(np.asarray(log_length_scale).reshape(-1)[0])))
    xs = np.asarray(X, dtype=np.float64).reshape(N, D) * s
    ys = np.asarray(Y, dtype=np.float64).reshape(M, D) * s
    x0, x1 = xs[:, 0], xs[:, 1]
    y0, y1 = ys[:, 0], ys[:, 1]
    one_n, zero_m, one_m = np.ones(N), np.zeros(M), np.ones(M)

    # X-side basis [6, N]: rows {1, x0, x1, x0*x1, x0^2, x1^2}
    L = np.stack([one_n, x0, x1, x0 * x1, x0 ** 2, x1 ** 2])

    # Y-side coefficient columns [6, M] per output channel
    c_dxdy = np.stack([y0 * y1, -y1, -y0, one_m, zero_m, zero_m])
    c_00 = np.stack([1 - y1 ** 2, zero_m, 2 * y1, zero_m, zero_m, -one_m])
    c_11 = np.stack([1 - y0 ** 2, 2 * y0, zero_m, zero_m, -one_m, zero_m])
    c_r2 = np.stack([y0 ** 2 + y1 ** 2, -2 * y0, -2 * y1, zero_m, one_m, one_m])

    Re = np.zeros((6, 2 * M))   # even output rows: [1-dy^2 | dxdy] interleaved
    Re[:, 0::2] = c_00
    Re[:, 1::2] = c_dxdy
    Ro = np.zeros((6, 2 * M))   # odd output rows: [dxdy | 1-dx^2] interleaved
    Ro[:, 0::2] = c_dxdy
    Ro[:, 1::2] = c_11

    # Merge Re/Ro into one tensor so each j-group is a single N=1024 matmul:
    # group g occupies cols [1024g, 1024g+1024) = [Re_g (512) | Ro_g (512)]
    Reo = np.zeros((6, 4 * M))
    v = Reo.reshape(6, 2 * M // 512, 2, 512)
    v[:, :, 0, :] = Re.reshape(6, -1, 512)
    v[:, :, 1, :] = Ro.reshape(6, -1, 512)

    Lh, Ll = _hi_lo(L)
    Lst = np.ascontiguousarray(np.concatenate([Lh, Ll, Lh], axis=0))  # (18, N)

    def r_stack(R):
        Rh, Rl = _hi_lo(R)
        return np.ascontiguousarray(np.concatenate([Rh, Rh, Rl], axis=0))

    return Lst, r_stack(Reo), r_stack(c_r2)


def _build_module(bass_cls=None, reps=1, **bass_kw):
    from concourse import bacc, mybir
    import concourse.tile as tile

    bf16 = mybir.dt.bfloat16
    f32 = mybir.dt.float32
    Exp = mybir.ActivationFunctionType.Exp

    if bass_cls is None:
        bass_cls = bacc.Bacc
    nc = bass_cls("TRN2", target_bir_lowering=False, debug=False,
                  enable_asserts=False, **bass_kw)
    lhsT_d = nc.dram_tensor("lhsT", [KST, NPC], bf16, kind="ExternalInput")
    # rr (M cols) and reo (4M cols) merged: one DMA, one semaphore wait
    rrreo_d = nc.dram_tensor("rrreo", [KST, 5 * M], bf16, kind="ExternalInput")
    out_d = nc.dram_tensor("out", [2 * NPC, 2 * M], f32, kind="ExternalOutput")

    QJ = 4 * JG  # 1024 j's covered by one r2/exp quad
    NQ = M // QJ  # 4 quads

    with tile.TileContext(nc) as tc:
        with (
            tc.tile_pool(name="const", bufs=1) as cpool,
            tc.tile_pool(name="outp", bufs=3) as opool,
            tc.tile_pool(name="ep", bufs=3) as epool,
            tc.tile_pool(name="ps", bufs=2, space="PSUM") as ppool,
        ):
            lhsT = cpool.tile([KST, NPC], bf16)
            nc.sync.dma_start(out=lhsT[:], in_=lhsT_d[:, :])
            rrreo = cpool.tile([KST, 5 * M], bf16)
            nc.sync.dma_start(out=rrreo[:], in_=rrreo_d[:, :])
            rr_sb = rrreo[:, 0:M]
            reo_sb = rrreo[:, M:5 * M]

            # Warm-up: ~3.5us of dense junk matmuls on lhsT while rrreo
            # loads, so the PE HAM clock-gate flips 1.2 -> 2.4 GHz before
            # real work. Results land in r2-tagged PSUM bufs, overwritten
            # later with start=True; never read.
            for w in range(4):
                junk = ppool.tile([IB, QJ], f32, tag="r2")
                for s in range(2):
                    nc.tensor.matmul(junk[:, s * 512:(s + 1) * 512],
                                     lhsT[:, 0:IB], lhsT[:, 0:NPC],
                                     start=True, stop=True)

            out_view = out_d.ap().rearrange("(i t) c -> i t c", t=2)

            for ib in [i for _ in range(reps) for i in range(NIB)]:
                wt = lhsT[:, ib * IB:(ib + 1) * IB]
                i0 = ib * IB
                for qp in range(NQ // 2):  # q-pair: unit of output DMA
                    # pair tile cols (h, q, j, t): a row-parity half h is a
                    # contiguous 16KB run per partition -> fat descriptors
                    oP = opool.tile([IB, 8 * QJ], f32, tag="oP")
                    oP5 = oP[:].rearrange("p (h q j t) -> p h q j t",
                                          h=2, q=2, t=2)
                    for qh in range(2):
                        q = 2 * qp + qh
                        r2q = ppool.tile([IB, QJ], f32, tag="r2")
                        for s in range(QJ // 512):
                            nc.tensor.matmul(
                                r2q[:, s * 512:(s + 1) * 512], wt,
                                rr_sb[:, q * QJ + s * 512:
                                      q * QJ + (s + 1) * 512],
                                start=True, stop=True)
                        ebig = epool.tile([IB, QJ], f32, tag="e")
                        nc.scalar.activation(ebig[:], r2q[:], Exp, scale=-0.5)
                        for h in range(QJ // JG):
                            g = q * (QJ // JG) + h
                            memo = ppool.tile([IB, 4 * JG], f32, tag="memo")
                            for s in range(4 * JG // 512):
                                nc.tensor.matmul(
                                    memo[:, s * 512:(s + 1) * 512], wt,
                                    reo_sb[:, g * 4 * JG + s * 512:
                                           g * 4 * JG + (s + 1) * 512],
                                    start=True, stop=True)
                            eb = (ebig[:, h * JG:(h + 1) * JG]
                                  .unsqueeze(1).unsqueeze(3)
                                  .broadcast_to([IB, 2, JG, 2]))
                            nc.vector.tensor_mul(
                                oP5[:, :, qh, h * JG:(h + 1) * JG, :],
                                memo[:].rearrange("p (h j t) -> p h j t",
                                                  h=2, t=2),
                                eb,
                            )
                    # two 2MB half-DMAs per pair, 16KB runs per partition
                    for hh in range(2):
                        nc.sync.dma_start(
                            out=out_view[i0:i0 + IB, hh:hh + 1,
                                         qp * 4 * QJ:(qp + 1) * 4 * QJ]
                            .squeeze(1),
                            in_=oP[:, hh * 4 * QJ:(hh + 1) * 4 * QJ])
    nc.finalize()
    return nc


def _run(X, Y, log_length_scale, trace=False):
    from concourse.bass_utils import run_bass_kernel_spmd

    Lst, Reo, Rr = _prepare_inputs(X, Y, log_length_scale)
    if "nc" not in _cache:
        _cache["nc"] = _build_module()
    nc = _cache["nc"]
    rrreo = np.ascontiguousarray(np.concatenate([Rr, Reo], axis=1))
    in_maps = [
        {
            "lhsT": np.ascontiguousarray(Lst[:, c * NPC:(c + 1) * NPC]),
            "rrreo": rrreo,
        }
        for c in range(NCORES)
    ]
    res = run_bass_kernel_spmd(nc, in_maps, core_ids=list(range(NCORES)),
                               trace=trace)
    out = np.concatenate([r["out"] for r in res.results], axis=0)
    return out.reshape(1, 2 * N, 2 * M), res


def kernel(X, Y, log_length_scale):
    out, _ = _run(np.asarray(X), np.asarray(Y), np.asarray(log_length_scale))
    return out

